# revision 1
# baseline (speedup 1.0000x reference)
"""Trainium2 Bass kernel for nn_LogicityPredictorVis.

The reference returns agg + x @ root + bias with shape [8, 4], which depends
ONLY on batch element 0 of every batched input (node_concepts[0], edge_attr[0],
batch_priorities[0]).  The B=4096 MLP sweep is dead code w.r.t. the output, so
the kernel computes just the batch-0 path.

Sharding: the NODE_CH=2048 contraction (node-MLP layer 3, the NNConv einsum,
and x @ root) is split over the 8 cores (256 channels each).  The small
replicated layers (node-MLP layers 1/2, edge MLP, pr layer 1) run on every
core.  Each core emits partial [8,4]-shaped results; the host sums them.

Einsum restructure: msg[k,o] = sum_c x[src_k,c] * w[k,c,o] with
w = (t @ pr_w2 + pr_b2) is rewritten by swapping the sums:
    msg[k,o] = sum_h t[k,h] * G[src_k,h,o] + xb[src_k,o]
    G[i,h,o] = sum_c x[i,c] * pr_w2[h, c*4+o]   (matmul, c-sharded)
    xb[i,o]  = sum_c x[i,c] * pr_b2[c*4+o]      (matmul, c-sharded)
so the only non-matmul step is one DVE multiply t[h,k'] * G[h,i,o] over
[128, 224].  The h-reduction uses the prod tensor itself as the matmul
STATIONARY operand with stride-4 column APs (out[k',o] = prod[:,o::4].T @
ones128), landing per-edge sums directly on PSUM partitions; the DST
aggregation, x@root, the pr_b2 term (via the complete-graph identity
sum_{k:DST=n} xb[src_k] = sum_i xb[i] - xb[n], i.e. (1-I) matmuls) and the
bias all accumulate into ONE [8,4] PSUM group feeding a single output DMA.

Inputs are packed on the host into per-partition-height f32 blobs ([128,*],
[64,*], [1,*]) laid out exactly as the SBUF images and loaded with six
concurrent dma_starts (queue concurrency IS the DMA bandwidth on this part),
staged so compute overlaps the stream.  Activations stay transposed ([C, 8],
channels on partitions) so every layer is matmul(lhsT=W_as_stored, rhs=prevT)
with no weight transposes; biases ride as K=1 matmuls against ones rows.  A
few tiny warm-up matmuls gated on the first 6 KB DMA pin the TensorE p-state
ramp near t=0.  Bacc's compile() legalizes sync waits for gen3 (each hardware
instruction carries at most one wait).
"""

import numpy as np

B, N = 4096, 8
C_IMG = 1024
NODE_CH = 2048
EDGE_CH = 3
ACT_CH = 4
E = N * (N - 1)
BBOX_MAX = 1024.0
N_CORES = 8
CS = NODE_CH // N_CORES        # 256 channels per core
C4O = CS * ACT_CH              # 1024 (c,o) pairs per core

_IDX = np.array([[i, j] for i in range(N) for j in range(N) if i != j],
                dtype=np.int32)
SRC = _IDX[:, 0]
DST = _IDX[:, 1]

# Three packed input tensors, grouped by partition count so narrow tensors
# don't pay for 128 partition rows of DMA.  name -> (partitions, free cols).
# w1 is stored m-major ((m, q, k) chunks) so layer-1 m-chunks can start as
# soon as their half of w1 has landed.
_B128 = [
    # early small tensors (edge MLP / pr deps), then x0T + w1 (split so the
    # first layer-1 m-chunks start before all of w1 lands), then the rest.
    ("pb1",    128, 1),
    ("ones128", 128, 1),
    ("ew2",    128, 2 * 64),
    ("rootpb", 128, 2 * 8),
    ("x0T",    128, 8 * N),
    ("w1",     128, 8 * 512),
    ("w2",     128, 4 * 256),
    ("w3",     128, 2 * CS),
    ("pw2pT",  128, ACT_CH * 2 * 128),
]
_B64 = [
    ("ew3",    64,  21),
    ("dselp",  56,  8),
    ("pw1r",   28,  128),
    ("maskblk", 28, 56),
    ("ew1",    8,   256),
    ("attrT",  8,   N),
    ("oneminusI", 8, 8),
    ("eye8",   8,   8),
]
_B1 = [
    ("b1rows", 1,   512),
    ("b2rows", 1,   256),
    ("b3rows", 1,   CS),
    ("eb1rows", 1,  256),
    ("eb2row", 1,   64),
    ("eb3row", 1,   21),
    ("p0row",  1,   8),
    ("ones8",  1,   8),
    ("biasrow4", 1, 4),
]

def _offsets(specs):
    offs, off = {}, 0
    for n, _p, c in specs:
        offs[n] = off
        off += c
    return offs, off

_OFF128, COLS128 = _offsets(_B128)
_OFF64, COLS64 = _offsets(_B64)
_OFF1, COLS1 = _offsets(_B1)
_SPEC = {n: ("b128", p, c, _OFF128[n]) for n, p, c in _B128}
_SPEC.update({n: ("b64", p, c, _OFF64[n]) for n, p, c in _B64})
_SPEC.update({n: ("b1", p, c, _OFF1[n]) for n, p, c in _B1})

_NC_CACHE = {}


def build_nc():
    """Build the per-core Bass program (identical on all cores)."""
    import concourse.bacc as bacc
    import concourse.mybir as mybir
    import concourse.tile as tile

    fp32 = mybir.dt.float32
    AF = mybir.ActivationFunctionType
    ALU = mybir.AluOpType

    nc = bacc.Bacc("TRN2", target_bir_lowering=False, debug=False)
    b128_d = nc.dram_tensor("b128", [128, COLS128], fp32,
                            kind="ExternalInput")
    b64_d = nc.dram_tensor("b64", [64, COLS64], fp32, kind="ExternalInput")
    b1_d = nc.dram_tensor("b1", [1, COLS1], fp32, kind="ExternalInput")
    outB_d = nc.dram_tensor("outB", [8, 4], fp32, kind="ExternalOutput")

    with tile.TileContext(nc) as tc:
        with tc.tile_pool(name="sb", bufs=1) as sb, \
             tc.tile_pool(name="ps", bufs=1, space="PSUM") as ps:

            # One SBUF tile per DMA stage so consumers only depend on the
            # stage that carries their tensor (a single shared tile would
            # serialize every consumer behind the last DMA).
            s1a_end = _OFF128["w1"] + 2048
            s1b_end = _OFF128["w2"]
            s2_end = _OFF128["pw2pT"]
            e_end = _OFF128["x0T"]
            b64_sb = sb.tile([64, COLS64], fp32, tag="b64")
            b1_sb = sb.tile([1, COLS1], fp32, tag="b1")
            tS1a = sb.tile([128, s1a_end], fp32, tag="tS1a")
            tS1b = sb.tile([128, s1b_end - s1a_end], fp32, tag="tS1b")
            tS2 = sb.tile([128, s2_end - s1b_end], fp32, tag="tS2")
            tS3 = sb.tile([128, COLS128 - s2_end], fp32, tag="tS3")
            # DMA order: tiny tensors first (unblock the edge/pr chain),
            # then x0T + w1 halves, then layers 2/3, then einsum weights.
            nc.sync.dma_start(b1_sb[:], b1_d[:])
            nc.sync.dma_start(b64_sb[:], b64_d[:])
            nc.sync.dma_start(tS1a[:], b128_d[:, 0:s1a_end])
            nc.sync.dma_start(tS1b[:], b128_d[:, s1a_end:s1b_end])
            nc.sync.dma_start(tS2[:], b128_d[:, s1b_end:s2_end])
            nc.sync.dma_start(tS3[:], b128_d[:, s2_end:])

            _t128 = [(0, tS1a), (s1a_end, tS1b),
                     (s1b_end, tS2), (s2_end, tS3)]

            def v(name):
                which, pp, cc, off = _SPEC[name]
                if which == "b64":
                    return b64_sb[0:pp, off:off + cc]
                if which == "b1":
                    return b1_sb[0:pp, off:off + cc]
                for base, t in reversed(_t128):
                    if off >= base:
                        assert off + cc <= base + t.shape[1], name
                        return t[0:pp, off - base:off - base + cc]
                raise KeyError(name)

            x0T_v = v("x0T").rearrange("p (q n) -> p q n", q=8)
            w1a_v = tS1a[:, _OFF128["w1"]:].rearrange(
                "p (m q k) -> p m q k", m=2, q=8)
            w1b_v = tS1b[:].rearrange("p (m q k) -> p m q k", m=2, q=8)
            w2_v = v("w2").rearrange("p (q m) -> p q m", q=4)
            w3_v = v("w3").rearrange("p (q m) -> p q m", q=2)
            ew2_v = v("ew2").rearrange("p (q m) -> p q m", q=2)
            pw2pT_v = tS3[:].rearrange("p (o q m) -> p o q m", o=4, q=2)
            rootpb_v = v("rootpb").rearrange("p (q m) -> p q m", q=2)
            b1rows_v, b2rows_v, b3rows_v = v("b1rows"), v("b2rows"), v("b3rows")
            pb1_v, attrT_v, ew1_v = v("pb1"), v("attrT"), v("ew1")
            eb1rows_v, eb2row_v = v("eb1rows"), v("eb2row")
            ew3_v, eb3row_v = v("ew3"), v("eb3row")
            p0row_v, pw1r_v, maskblk_v = v("p0row"), v("pw1r"), v("maskblk")
            dselp_v = v("dselp")
            oneminusI_v, eye8_v = v("oneminusI"), v("eye8")
            ones8_v, ones128_v = v("ones8"), v("ones128")
            biasrow4_v = v("biasrow4")

            # PE warm-up: a few tiny matmuls gated only on the first (6 KB)
            # DMA pin the TensorE busy-ramp start near t=0, so the p-state is
            # at full clock when the real matmuls arrive (the engine ramps
            # after ~3us of busy time; without this every matmul in the
            # DMA-shadowed MLP runs throttled).
            p_warm = ps.tile([1, 128], fp32, tag="ps_w", bufs=1)
            for _wi in range(8):
                nc.tensor.matmul(p_warm[:], ones8_v[0:1, 0:1],
                                 b1_sb[0:1, 0:128],
                                 start=True, stop=True, skip_group_check=True)

            # ---------- node MLP (transposed activations [C, 8]) ----------
            # Bias rides as a K=1 matmul so one ACT op finishes each layer.
            p_h1 = ps.tile([128, 4, N], fp32, tag="ps_n", bufs=2)
            for m in range(4):
                w1mv = w1a_v[:, m, :, :] if m < 2 else w1b_v[:, m - 2, :, :]
                nc.tensor.matmul(p_h1[:, m, :],
                                 b1rows_v[:, m * 128:(m + 1) * 128],
                                 ones8_v, start=True, stop=False,
                                 skip_group_check=True)
                for q in range(8):
                    nc.tensor.matmul(p_h1[:, m, :], w1mv[:, q, :],
                                     x0T_v[:, q, :], start=False,
                                     stop=(q == 7), skip_group_check=True)
            h1T_sb = sb.tile([128, 4, N], fp32, tag="h1T")
            nc.scalar.activation(h1T_sb[:], p_h1[:], AF.Relu)

            p_h2 = ps.tile([128, 2, N], fp32, tag="ps_n", bufs=2)
            for m in range(2):
                nc.tensor.matmul(p_h2[:, m, :],
                                 b2rows_v[:, m * 128:(m + 1) * 128],
                                 ones8_v, start=True, stop=False,
                                 skip_group_check=True)
                for q in range(4):
                    nc.tensor.matmul(p_h2[:, m, :],
                                     w2_v[:, q, m * 128:(m + 1) * 128],
                                     h1T_sb[:, q, :], start=False,
                                     stop=(q == 3), skip_group_check=True)
            h2T_sb = sb.tile([128, 2, N], fp32, tag="h2T")
            nc.scalar.activation(h2T_sb[:], p_h2[:], AF.Relu)

            # layer 3 (c-sharded): xT[c, i], plain layout.
            p_x = ps.tile([128, 2, N], fp32, tag="ps_n", bufs=2)
            for m in range(2):
                nc.tensor.matmul(p_x[:, m, :],
                                 b3rows_v[:, m * 128:(m + 1) * 128],
                                 ones8_v, start=True, stop=False,
                                 skip_group_check=True)
                for q in range(2):
                    nc.tensor.matmul(p_x[:, m, :],
                                     w3_v[:, q, m * 128:(m + 1) * 128],
                                     h2T_sb[:, q, :], start=False,
                                     stop=(q == 1), skip_group_check=True)
            xT_sb = sb.tile([128, 2, N], fp32, tag="xT")
            nc.scalar.activation(xT_sb[:], p_x[:], AF.Sigmoid)

            # ---------- edge MLP (transposed) ----------
            p_g1 = ps.tile([128, 2, N], fp32, tag="ps_e", bufs=2)
            for m in range(2):
                nc.tensor.matmul(p_g1[:, m, :],
                                 eb1rows_v[:, m * 128:(m + 1) * 128],
                                 ones8_v, start=True, stop=False,
                                 skip_group_check=True)
                nc.tensor.matmul(p_g1[:, m, :],
                                 ew1_v[:, m * 128:(m + 1) * 128],
                                 attrT_v, start=False, stop=True,
                                 skip_group_check=True)
            g1T_sb = sb.tile([128, 2, N], fp32, tag="g1T")
            nc.scalar.activation(g1T_sb[:], p_g1[:], AF.Relu)

            p_g2 = ps.tile([64, N], fp32, tag="ps_e", bufs=2)
            for q in range(2):
                nc.tensor.matmul(p_g2[:], ew2_v[:, q, :], g1T_sb[:, q, :],
                                 start=(q == 0), stop=False,
                                 skip_group_check=True)
            nc.tensor.matmul(p_g2[:], eb2row_v, ones8_v, start=False,
                             stop=True, skip_group_check=True)
            g2T_sb = sb.tile([64, N], fp32, tag="g2T")
            nc.scalar.activation(g2T_sb[:], p_g2[:], AF.Relu)

            # ea node-major: ea[i, j'*3+ch]
            p_ea = ps.tile([8, 21], fp32, tag="ps_e", bufs=2)
            nc.tensor.matmul(p_ea[:], g2T_sb[:], ew3_v, start=True,
                             stop=False, skip_group_check=True)
            nc.tensor.matmul(p_ea[:], ones8_v, eb3row_v, start=False,
                             stop=True, skip_group_check=True)
            ean_sb = sb.tile([8, 21], fp32, tag="ean")
            nc.scalar.activation(ean_sb[:], p_ea[:], AF.Sigmoid)

            # ---------- HigherPri channel ----------
            p_pc = ps.tile([8, 8], fp32, tag="ps_e", bufs=2)
            nc.tensor.matmul(p_pc[:], p0row_v, ones8_v, start=True, stop=True)
            p_pr = ps.tile([8, 8], fp32, tag="ps_e", bufs=2)
            nc.tensor.matmul(p_pr[:], ones8_v, p0row_v, start=True, stop=True)
            pc_sb = sb.tile([8, 8], fp32, tag="pc")
            pr_sb = sb.tile([8, 8], fp32, tag="pr")
            nc.vector.tensor_copy(pc_sb[:], p_pc[:])
            nc.vector.tensor_copy(pr_sb[:], p_pr[:])
            hp_sb = sb.tile([8, 8], fp32, tag="hp")
            nc.vector.tensor_tensor(hp_sb[:], pc_sb[:], pr_sb[:], op=ALU.is_gt)

            # ---------- e by node, then one PE transpose ----------
            q4_sb = sb.tile([8, 7, 4], fp32, tag="q4")
            nc.vector.tensor_copy(q4_sb[:, :, 0:3],
                                  ean_sb[:].rearrange("i (j c) -> i j c", c=3))
            nc.vector.tensor_copy(q4_sb[:, :, 3], hp_sb[:, 0:7])
            p_q4T = ps.tile([28, 8], fp32, tag="ps_e", bufs=2)
            nc.tensor.transpose(p_q4T[:],
                                q4_sb[:].rearrange("i j c -> i (j c)"),
                                eye8_v)
            q4T_sb = sb.tile([28, 8], fp32, tag="q4T")   # [(j'*4+ch), i]
            nc.vector.tensor_copy(q4T_sb[:], p_q4T[:])

            # ---------- pr layer 1: block-diagonal rhs, one K=28 matmul ----
            rhs2_sb = sb.tile([28, E], fp32, tag="rhs2")
            nc.vector.tensor_tensor(
                rhs2_sb[:].rearrange("p (j i) -> p j i", i=8),
                q4T_sb[:].unsqueeze(1).broadcast_to([28, 7, N]),
                maskblk_v.rearrange("p (j i) -> p j i", i=8),
                op=ALU.mult)
            p_t = ps.tile([128, E], fp32, tag="ps_e", bufs=2)
            nc.tensor.matmul(p_t[:], pw1r_v, rhs2_sb[:], start=True,
                             stop=True)
            tT_sb = sb.tile([128, E], fp32, tag="tT")    # [h, j'*8+i]
            nc.scalar.activation(tT_sb[:], p_t[:], AF.Relu, bias=pb1_v)

            # ---------- G[i,h,o] = sum_c x[i,c] pw2[h,c4o]  (c-sharded) ----
            p_G = ps.tile([128, 4, N], fp32, tag="ps_t2", bufs=3)
            for o in range(4):
                for q in range(2):
                    nc.tensor.matmul(p_G[:, o, :], pw2pT_v[:, o, q, :],
                                     xT_sb[:, q, :], start=(q == 0),
                                     stop=(q == 1), skip_group_check=True)
            # prod2[h, (j',i,o)] = t[h, j'*8+i] * G[h, i, o]
            # (in1 reads the G PSUM bank directly - DVE may read PSUM)
            prod2_sb = sb.tile([128, 7 * N * 4], fp32, tag="prod2")
            nc.vector.tensor_tensor(
                prod2_sb[:].rearrange("p (j i o) -> p j i o", i=8, o=4),
                tT_sb[:].rearrange("p (j i) -> p j i", i=8)
                        .broadcast_to([128, 7, N, 4]),
                p_G[:].rearrange("p o i -> p i o").unsqueeze(1)
                      .broadcast_to([128, 7, N, 4]),
                op=ALU.mult)
            # reduce over h straight into per-edge partitions: prod2's
            # stride-4 o-columns as the STATIONARY operand give
            # s4[k', o] = sum_h prod2[h, (k',o)] in one matmul per o.
            p_s4 = ps.tile([56, 4], fp32, tag="ps_t2", bufs=3)
            for o in range(4):
                nc.tensor.matmul(p_s4[:, o:o + 1], prod2_sb[:, o:224:4],
                                 ones128_v, start=True, stop=True,
                                 skip_group_check=True)
            s4_sb = sb.tile([56, 4], fp32, tag="s4")
            nc.vector.tensor_copy(s4_sb[:], p_s4[:])

            # ---------- one accumulation: x@root + xb + msg-agg + bias -----
            p_o2 = ps.tile([8, 8], fp32, tag="ps_t2", bufs=3)
            for q in range(2):
                nc.tensor.matmul(p_o2[:], xT_sb[:, q, :], rootpb_v[:, q, :],
                                 start=(q == 0), stop=(q == 1),
                                 skip_group_check=True)
            o2_sb = sb.tile([8, 8], fp32, tag="o2")
            nc.vector.tensor_copy(o2_sb[:], p_o2[:])
            p_o3 = ps.tile([8, 4], fp32, tag="ps_t2", bufs=3)
            nc.tensor.matmul(p_o3[:], ones8_v, biasrow4_v, start=True,
                             stop=False, skip_group_check=True)
            nc.tensor.matmul(p_o3[:], eye8_v, o2_sb[:, 0:4], start=False,
                             stop=False, skip_group_check=True)
            nc.tensor.matmul(p_o3[:], oneminusI_v, o2_sb[:, 4:8],
                             start=False, stop=False, skip_group_check=True)
            nc.tensor.matmul(p_o3[:], dselp_v, s4_sb[:], start=False,
                             stop=True, skip_group_check=True)
            o3_sb = sb.tile([8, 4], fp32, tag="o3")
            nc.vector.tensor_copy(o3_sb[:], p_o3[:])
            nc.sync.dma_start(outB_d[:], o3_sb[:])

    nc.compile()
    return nc


def _chunked(x, q):
    """[q*128, m] -> [128, q*m] image (partition p holds chunk-major rows)."""
    q128, m = x.shape
    assert q128 == q * 128
    return x.reshape(q, 128, m).transpose(1, 0, 2).reshape(128, q * m)


def make_in_maps(inputs):
    """Host-side sharding: build the per-core packed blobs (numpy glue)."""
    f = np.float32

    def a(x):
        return np.ascontiguousarray(np.asarray(x, dtype=f))

    roi = a(inputs["roi_features"][0])
    bbox = a(inputs["batch_bboxes"][0])
    dirs = a(inputs["batch_directions"][0])
    p0 = a(inputs["batch_priorities"][0])

    base = {"b128": np.zeros((128, COLS128), f),
            "b64": np.zeros((64, COLS64), f),
            "b1": np.zeros((1, COLS1), f)}

    def put(dst, name, img):
        which, pp, cc, off = _SPEC[name]
        img = np.asarray(img, f)
        assert img.shape == (pp, cc), (name, img.shape, (pp, cc))
        dst[which][0:pp, off:off + cc] = img

    put(base, "x0T", _chunked(a(roi.T), 8))
    # w1 image m-major: [p, (m, q, k)] = w1[q*128+p, m*128+k]
    w1 = a(inputs["ncp_w1"]).reshape(8, 128, 4, 128)
    put(base, "w1", np.ascontiguousarray(w1.transpose(1, 2, 0, 3))
        .reshape(128, 4096))
    put(base, "w2", _chunked(a(inputs["ncp_w2"]), 4))
    put(base, "b1rows", a(inputs["ncp_b1"]).reshape(1, 512))
    put(base, "b2rows", a(inputs["ncp_b2"]).reshape(1, 256))
    put(base, "pb1", a(inputs["pr_b1"]).reshape(128, 1))
    put(base, "attrT", np.concatenate([bbox / BBOX_MAX, dirs], axis=1).T)
    put(base, "ew1", a(inputs["ep_w1"]))
    put(base, "eb1rows", a(inputs["ep_b1"]).reshape(1, 256))
    put(base, "ew2", _chunked(a(inputs["ep_w2"]), 2))
    put(base, "eb2row", a(inputs["ep_b2"]).reshape(1, 64))
    put(base, "ew3", a(inputs["ep_w3"]))
    put(base, "eb3row", a(inputs["ep_b3"]).reshape(1, 21))
    put(base, "p0row", p0.reshape(1, 8))
    put(base, "pw1r", np.tile(a(inputs["pr_w1"]), (7, 1)))
    mb = np.zeros((28, 56), f)
    for jp in range(7):
        mb[jp * 4:(jp + 1) * 4, jp * 8:(jp + 1) * 8] = 1.0
    put(base, "maskblk", mb)
    # DST selector on the k' = j'*8 + i edge axis
    dselp = np.zeros((E, 8), f)
    for jp in range(7):
        for i in range(N):
            dselp[jp * 8 + i, DST[i * 7 + jp]] = 1.0
    put(base, "dselp", dselp)
    put(base, "oneminusI", np.ones((8, 8), f) - np.eye(8, dtype=f))
    put(base, "eye8", np.eye(8, dtype=f))
    put(base, "ones8", np.ones((1, 8), f))
    put(base, "ones128", np.ones((128, 1), f))

    w3_full = a(inputs["ncp_w3"])
    b3_full = a(inputs["ncp_b3"])
    pw2_full = a(inputs["pr_w2"])
    pb2_full = a(inputs["pr_b2"])
    root_full = a(inputs["root"])
    bias = a(inputs["bias"]).reshape(ACT_CH)

    in_maps = []
    for j in range(N_CORES):
        cs = slice(j * CS, (j + 1) * CS)
        c4s = slice(j * C4O, (j + 1) * C4O)
        blob = {k: b.copy() for k, b in base.items()}
        put(blob, "w3", _chunked(np.ascontiguousarray(w3_full[:, cs]), 2))
        put(blob, "b3rows", b3_full[cs].reshape(1, CS))
        # pw2pT[p, (o, q, h)] = pw2[h, (q*128+p)*4 + o]
        t = pw2_full[:, c4s].reshape(128, 2, 128, ACT_CH)   # (h, q, p, o)
        put(blob, "pw2pT",
            np.ascontiguousarray(t.transpose(2, 3, 1, 0)).reshape(128, -1))
        rootpb = np.concatenate(
            [root_full[cs], pb2_full[c4s].reshape(CS, ACT_CH)], axis=1)
        put(blob, "rootpb", _chunked(rootpb, 2))
        put(blob, "biasrow4",
            bias.reshape(1, 4) if j == 0 else np.zeros((1, 4), f))
        in_maps.append(blob)
    return in_maps


def kernel(**inputs):
    from concourse.bass_utils import run_bass_kernel_spmd

    if "nc" not in _NC_CACHE:
        _NC_CACHE["nc"] = build_nc()
    nc = _NC_CACHE["nc"]
    in_maps = make_in_maps(inputs)
    res = run_bass_kernel_spmd(nc, in_maps, list(range(N_CORES)))
    tot = np.zeros((8, 4), np.float32)
    for r in res.results:
        tot += np.asarray(r["outB"], np.float32)
    return tot

